# revision 6
# baseline (speedup 1.0000x reference)
"""FP8 MoE (top-2, 8 experts) Trainium2 kernel.

Strategy (expert-parallel over 8 NeuronCores):
  - Host: compute per-token per-expert gate = sum(routing_weights where
    selected_experts == e).  Tokens with gate == 0 contribute exactly 0 to the
    reference output, so each expert only processes its selected tokens
    (~T*K/E each instead of T).
  - Host: quantize activations x -> fp8 e4m3fn exactly as the reference does,
    then map the e4m3fn grid onto Trainium's IEEE e4m3 grid (max 240 vs 448)
    by halving (exact: exponent decrement).  Weights (already e4m3fn values
    stored as f32) are halved the same way.  The compensating 4x lands in the
    dequant scales.
  - Device (per core = per expert): h~ = xq_t @ w1_t^T via DoubleRow fp8
    matmuls (features on PSUM partitions, tokens on the moving free dim);
    aq = fp8(clip(silu(s1*g~) * c2*u~, +-224)); y~ = aq_t @ w2_t^T with
    DoubleRow.  DR adds ~1e-4 noise; total rel_fro ~3e-3, well inside the
    2e-2 gate.
  - Tokens live in <=504-wide column slots at 512-aligned offsets: 512-wide
    matmuls (moving free dim 2*512 = fp8 max, full PSUM bank) measure ~8ns/MM
    slower than narrower ones, and 512-aligned slot offsets keep the fp8
    DoubleRow pair-stride 16B-aligned without rounding the token capacity up.
  - DMA: xq rides the Scalar (Activation) HWDGE queue in parallel with the
    weight stream on the Sync queue; the per-partition-tiny sc transfer goes
    last so it cannot clog the prologue.
  - Host: out[tok] += gate * s2 * y~  (s2 = 4*iscale2*wscale2), experts in
    ascending order like the reference loop.
"""

import os

import numpy as np
import ml_dtypes

import concourse.mybir as mybir
from concourse import bacc
from concourse.tile import TileContext
from concourse.bass_utils import run_bass_kernel_spmd

FP8_MAX = 448.0
E4 = mybir.dt.float8e4
F32 = mybir.dt.float32
E4NP = ml_dtypes.float8_e4m3        # TRN fp8 (IEEE, max 240)
FNNP = ml_dtypes.float8_e4m3fn      # OCP fp8 (max 448) — the reference format

# Problem sizes (hardcoded; harness contract).
T, H, I, E, TOPK = 4096, 2048, 4096, 8, 2

SLOT_W = 504        # max tokens per column slot (see module docstring)
SLOT_STRIDE = 512   # slot column offsets are 512-aligned

# Module global holding the most recent BassKernelResults (for test harness).
LAST_RESULT = None

# Compiled-kernel cache keyed by the slot layout so repeated kernel() calls
# with the same routing shape skip the neuronxcc compile.
_NC_CACHE = {}


# --------------------------------------------------------------------------
# Device kernel builder (shape-generic so it can be smoke-tested small).
# --------------------------------------------------------------------------

def build_nc(CP, slots, h=H, i_dim=I, n_cores=8, mm1_dr=True, mm2_dr=True):
    """Two fp8 GEMMs + SiLU-gate epilogue for one expert.

    Tokens occupy column slots [off, off+sz) of a CP-wide layout.

    Inputs (per core):
      xq  [KT1, 128, 2*CP] fp8e4 : xq^T tiled (k-chunk, pair, token-col)
      w1p [M1, 128, KT1*256] fp8e4 : w1^T tiled per output-feature block
      w2p [M2, 128, KT2*256] fp8e4
      sc  [128, 2] f32 : col0 = s1 = 4*is1*ws1, col1 = c2 = s1/(2*is2)
    Output:
      yT  [h, CP] f32 : y~^T (caller applies s2 = 4*is2*ws2 and the gate)
    """
    assert h % 256 == 0 and i_dim % 256 == 0
    assert CP % 16 == 0
    nslices = [slice(off, off + sz) for off, sz in slots]
    for off, sz in slots:
        assert off % 16 == 0 and 0 < sz <= 512 and off + sz <= CP
    KT1 = h // 256          # mm1 k-chunks (256 deep each w/ DoubleRow)
    KT2 = i_dim // 256      # mm2 k-chunks
    MG = i_dim // 128       # gate-half feature tiles (m and m+MG pair up)
    M1 = 2 * MG             # w1 output tiles
    M2 = h // 128           # w2 output tiles
    DR = mybir.MatmulPerfMode.DoubleRow
    mult = mybir.AluOpType.mult

    nc = bacc.Bacc("TRN2", target_bir_lowering=False, debug=False,
                   num_devices=n_cores)
    xq = nc.dram_tensor("xq", [KT1, 128, 2 * CP], E4, kind="ExternalInput")
    w1p = nc.dram_tensor("w1p", [M1, 128, KT1 * 256], E4, kind="ExternalInput")
    w2p = nc.dram_tensor("w2p", [M2, 128, KT2 * 256], E4, kind="ExternalInput")
    sc = nc.dram_tensor("sc", [128, 2], F32, kind="ExternalInput")
    yT = nc.dram_tensor("yT", [h, CP], F32, kind="ExternalOutput")

    with TileContext(nc) as tc:
        with (
            tc.tile_pool(name="cpool", bufs=1) as cpool,
            tc.tile_pool(name="xqpool", bufs=1) as xqpool,
            tc.tile_pool(name="aqpool", bufs=1) as aqpool,
            tc.tile_pool(name="w1pool", bufs=6) as w1pool,
            tc.tile_pool(name="w2pool", bufs=4) as w2pool,
            tc.tile_pool(name="eppool", bufs=3) as eppool,
            tc.tile_pool(name="ypool", bufs=4) as ypool,
            tc.tile_pool(name="psA", bufs=2, space="PSUM") as psA,
            tc.tile_pool(name="psB", bufs=2, space="PSUM") as psB,
            tc.tile_pool(name="psY", bufs=3, space="PSUM") as psY,
        ):
            # PE warmup: dependency-free matmuls on a memset tile start the
            # HAM clock ramp while the first input DMAs are in flight.  Kept
            # short: the first real matmul can start as soon as xq chunk 0
            # lands (~8 us), and in-order PE would stall behind extra warmup.
            wrm = cpool.tile([128, 512], E4, name="wrm")
            nc.vector.memset(wrm, 0)
            pwrm = psY.tile([128, 512], F32, name="pwrm", bufs=1, tag="pwrm")
            for _ in range(3):
                nc.tensor.matmul(pwrm, wrm[:, 0:128], wrm, start=True,
                                 stop=True)

            def load_w1(j):
                wt = w1pool.tile([128, KT1 * 256], E4, name=f"w1t{j}",
                                 tag="w1")
                nc.sync.dma_start(out=wt, in_=w1p.ap()[j])
                return wt

            # Prologue schedule: the first matmul group needs w1 tiles
            # (j=0, MG) plus ALL xq k-chunks.  Split those across the two
            # HWDGE queues so they stream in parallel: Sync carries the two
            # w1 tiles + the tail xq chunks, Scalar carries the head xq
            # chunks.  The sc transfer (128 rows x 8 B = tiny packets) goes
            # last on Scalar where it cannot delay anything.
            w1_first = (load_w1(0), load_w1(MG))
            xq_ts = []
            n_scalar_xq = (5 * KT1) // 8
            for k in range(KT1):
                xqk = xqpool.tile([128, 2, CP], E4, name=f"xqk{k}",
                                  tag=f"xqk{k}")
                eng = nc.scalar if k < n_scalar_xq else nc.sync
                eng.dma_start(out=xqk, in_=xq.ap()[k].rearrange(
                    "p (i n) -> p i n", i=2))
                xq_ts.append(xqk)
            sc_t = cpool.tile([128, 2], F32, name="sc_t")
            nc.scalar.dma_start(out=sc_t, in_=sc.ap())
            s1_ap = sc_t[:, 0:1]
            c2_ap = sc_t[:, 1:2]
            aq_t = aqpool.tile([128, 2 * KT2, CP], E4, name="aq_t")

            # ---- mm1 + gated epilogue: aq^T[i_dim, CP] in fp8 ----
            def mm1_epilogue(jg, nsl, pg, pu):
                nt_sz = nsl.stop - nsl.start
                tg = eppool.tile([128, nt_sz], F32, name="tg", tag="tg")
                nc.scalar.activation(tg, pg,
                                     mybir.ActivationFunctionType.Silu,
                                     scale=s1_ap)
                v = eppool.tile([128, nt_sz], F32, name="v", tag="v")
                nc.vector.scalar_tensor_tensor(v, pu, c2_ap, tg,
                                               op0=mult, op1=mult)
                nc.vector.tensor_scalar(
                    aq_t[:, jg, nsl], v, 224.0, -224.0,
                    op0=mybir.AluOpType.min, op1=mybir.AluOpType.max)

            # jg = 0 runs k-major: each xq chunk feeds 4 matmuls (2 halves x
            # 2 slots, interleaved PSUM accumulation groups) the moment it
            # lands, so the PE tracks the xq DMA stream instead of idling
            # until the whole 2 MB transfer completes.
            if mm1_dr is True and len(nslices) <= 2:
                wg, wu = w1_first
                pgs = [psA.tile([128, s.stop - s.start], F32, name="pg",
                                tag="pg") for s in nslices]
                pus = [psB.tile([128, s.stop - s.start], F32, name="pu",
                                tag="pu") for s in nslices]
                for k in range(KT1):
                    for wtile, ptiles in ((wg, pgs), (wu, pus)):
                        lh = wtile[:, k * 256:(k + 1) * 256].rearrange(
                            "p (i m) -> p i m", i=2)
                        for nsl, ptile in zip(nslices, ptiles):
                            nc.tensor.matmul(
                                ptile, lh, xq_ts[k][:, :, nsl],
                                start=(k == 0), stop=(k == KT1 - 1),
                                perf_mode=DR)
                for nsl, pg, pu in zip(nslices, pgs, pus):
                    mm1_epilogue(0, nsl, pg, pu)
                jg_start = 1
            else:
                jg_start = 0

            for jg in range(jg_start, MG):
                wg, wu = (w1_first if jg == 0
                          else (load_w1(jg), load_w1(jg + MG)))
                for nsl in nslices:
                    nt_sz = nsl.stop - nsl.start
                    pg = psA.tile([128, nt_sz], F32, name="pg", tag="pg")
                    pu = psB.tile([128, nt_sz], F32, name="pu", tag="pu")
                    for half, (wtile, ptile) in enumerate(((wg, pg),
                                                          (wu, pu))):
                        use_dr = (mm1_dr is True
                                  or (mm1_dr == "g" and half == 0)
                                  or (mm1_dr == "u" and half == 1))
                        if use_dr:
                            for k in range(KT1):
                                lh = wtile[:, k * 256:(k + 1) * 256].rearrange(
                                    "p (i m) -> p i m", i=2)
                                rx = xq_ts[k][:, :, nsl]
                                nc.tensor.matmul(
                                    ptile, lh, rx, start=(k == 0),
                                    stop=(k == KT1 - 1), perf_mode=DR)
                        else:
                            for c in range(2 * KT1):
                                lh = wtile[:, c * 128:(c + 1) * 128]
                                rx = xq_ts[c // 2][:, c % 2, nsl]
                                nc.tensor.matmul(
                                    ptile, lh, rx, start=(c == 0),
                                    stop=(c == 2 * KT1 - 1))
                    mm1_epilogue(jg, nsl, pg, pu)

            # ---- mm2: y~^T[h, CP] ----
            for m in range(M2):
                w2t = w2pool.tile([128, KT2 * 256], E4, name="w2t", tag="w2")
                nc.sync.dma_start(out=w2t, in_=w2p.ap()[m])
                for nsl in nslices:
                    nt_sz = nsl.stop - nsl.start
                    py = psY.tile([128, nt_sz], F32, name="py", tag="py")
                    if mm2_dr:
                        for k in range(KT2):
                            lw = w2t[:, k * 256:(k + 1) * 256].rearrange(
                                "p (i m) -> p i m", i=2)
                            ra = aq_t[:, 2 * k:2 * k + 2, nsl]
                            nc.tensor.matmul(py, lw, ra, start=(k == 0),
                                             stop=(k == KT2 - 1), perf_mode=DR)
                    else:
                        for c in range(2 * KT2):
                            lw = w2t[:, c * 128:(c + 1) * 128]
                            ra = aq_t[:, c, nsl]
                            nc.tensor.matmul(py, lw, ra, start=(c == 0),
                                             stop=(c == 2 * KT2 - 1))
                    yt = ypool.tile([128, nt_sz], F32, name="yt", tag="yt")
                    if m == M2 - 1:
                        # tail: split copy+store into halves on both HWDGE
                        # queues so the last DMA chases a half-size copy.
                        hh = (nt_sz // 2 + 7) // 8 * 8
                        for qeng, hsl in ((nc.scalar, slice(0, hh)),
                                          (nc.sync, slice(hh, nt_sz))):
                            nc.vector.tensor_copy(out=yt[:, hsl],
                                                  in_=py[:, hsl])
                            qeng.dma_start(
                                out=yT.ap()[m * 128:(m + 1) * 128,
                                            nsl.start + hsl.start:
                                            nsl.start + hsl.stop],
                                in_=yt[:, hsl])
                    else:
                        nc.vector.tensor_copy(out=yt, in_=py)
                        nc.sync.dma_start(
                            out=yT.ap()[m * 128:(m + 1) * 128, nsl], in_=yt)
    nc.compile()
    return nc


# --------------------------------------------------------------------------
# Host-side packing
# --------------------------------------------------------------------------

def _halve_to_trn(q_fn_f32):
    """e4m3fn values (held in f32) -> TRN e4m3 at half scale (exact)."""
    return (q_fn_f32.astype(np.float32) * 0.5).astype(E4NP)


def pack_w1(w1_e, h, i_dim):
    """w1_e [2I, H] f32 (e4m3fn values) -> [M1, 128, KT1*256] TRN fp8."""
    M1, KT1 = (2 * i_dim) // 128, h // 256
    q = _halve_to_trn(w1_e)
    t = q.reshape(M1, 128, KT1, 2, 128)            # [m, mm, k, i, p]
    t = np.ascontiguousarray(t.transpose(0, 4, 2, 3, 1))  # [m, p, k, i, mm]
    return t.reshape(M1, 128, KT1 * 256)


def pack_w2(w2_e, h, i_dim):
    """w2_e [H, I] f32 (e4m3fn values) -> [M2, 128, KT2*256] TRN fp8."""
    M2, KT2 = h // 128, i_dim // 256
    q = _halve_to_trn(w2_e)
    t = q.reshape(M2, 128, KT2, 2, 128)
    t = np.ascontiguousarray(t.transpose(0, 4, 2, 3, 1))
    return t.reshape(M2, 128, KT2 * 256)


def quantize_ref(xg, iscale):
    """Exactly the reference's _to_fp8(x/iscale), values in f32."""
    q = np.clip(xg.astype(np.float32) / iscale, -FP8_MAX, FP8_MAX)
    return q.astype(FNNP).astype(np.float32)


def pack_xq(xq_fn_f32, CP, colmap, h):
    """Quantized tokens [cnt, H] (e4m3fn values) -> [KT1, 128, 2*CP]."""
    KT1 = h // 256
    cnt = xq_fn_f32.shape[0]
    zq = np.zeros((CP, h), dtype=E4NP)
    zq[colmap[:cnt]] = _halve_to_trn(xq_fn_f32)
    xqT = np.ascontiguousarray(zq.T)               # [h, CP]
    t = xqT.reshape(KT1, 2, 128, CP)               # [k, i, p, n]
    t = np.ascontiguousarray(t.transpose(0, 2, 1, 3))  # [k, p, i, n]
    return t.reshape(KT1, 128, 2 * CP)


def choose_capacity(max_cnt):
    """Slot layout for max_cnt tokens: (C, CP, slots, colmap).

    slots are (offset, width) with width <= SLOT_W and 512-aligned offsets;
    colmap[i] is the column index of the i-th packed token.
    """
    C = max(max_cnt, 8)
    nslot = -(-C // SLOT_W)
    slots = []
    left = C
    for i in range(nslot):
        w = min(SLOT_W, left)
        slots.append((SLOT_STRIDE * i, w))
        left -= w
    CP = -(-(slots[-1][0] + slots[-1][1]) // 16) * 16
    colmap = np.concatenate([np.arange(off, off + w) for off, w in slots])
    return C, CP, slots, colmap


def _maybe_enable_trace():
    """NTFF tracing (MOE_TRACE=1): install the antenv.axon_hooks shim this
    image lacks so run_bass_kernel_spmd(trace=True) works under axon."""
    if not os.environ.get("MOE_TRACE"):
        return False
    try:
        import antenv.axon_hooks  # noqa: F401
    except ImportError:
        import sys
        import types
        mod = types.ModuleType("antenv.axon_hooks")
        mod._hook = None
        mod.set_axon_ntff_profile_hook = lambda h: setattr(mod, "_hook", h)
        mod.get_axon_ntff_profile_hook = lambda: mod._hook
        sys.modules["antenv.axon_hooks"] = mod
        try:
            from trn_agent_boot.trn_boot import _ntff_profile_via_ctypes
            mod._hook = _ntff_profile_via_ctypes("/opt/axon/libaxon_pjrt.so")
        except Exception:
            return False
    return True


# --------------------------------------------------------------------------
# Entry point
# --------------------------------------------------------------------------

def kernel(x, selected_experts, routing_weights, w1, w2,
           w1_iscale, w2_iscale, w1_wscale, w2_wscale):
    global LAST_RESULT
    x = np.asarray(x)
    sel = np.asarray(selected_experts)
    rw = np.asarray(routing_weights).astype(np.float32)
    w1 = np.asarray(w1)
    w2 = np.asarray(w2)
    w1_iscale = np.asarray(w1_iscale, dtype=np.float32)
    w2_iscale = np.asarray(w2_iscale, dtype=np.float32)
    w1_wscale = np.asarray(w1_wscale, dtype=np.float32)
    w2_wscale = np.asarray(w2_wscale, dtype=np.float32)

    t_dim = x.shape[0]
    # gate[t, e] = sum_k rw[t, k] * (sel[t, k] == e)
    gate = np.zeros((t_dim, E), dtype=np.float32)
    rows = np.arange(t_dim)
    for kk in range(sel.shape[1]):
        np.add.at(gate, (rows, sel[:, kk]), rw[:, kk])

    idxs = [np.flatnonzero(gate[:, e] != 0.0) for e in range(E)]
    counts = [len(ix) for ix in idxs]
    # SBUF caps the per-round token capacity; very skewed routing falls back
    # to multiple rounds over token chunks (key-0 inputs need one round).
    MAX_C = 2016
    rounds = max(1, -(-max(counts) // MAX_C))
    chunked = [np.array_split(ix, rounds) for ix in idxs]
    C, CP, slots, colmap = choose_capacity(
        max(len(ch) for chs in chunked for ch in chs))

    mm1_dr = {"1": True, "g": "g", "u": "u", "0": False}.get(
        os.environ.get("MOE_MM1_DR", ""), True)
    key = (CP, tuple(slots), mm1_dr)
    nc = _NC_CACHE.get(key)
    if nc is None:
        nc = _NC_CACHE[key] = build_nc(CP, slots, mm1_dr=mm1_dr)
    w1_packed = [pack_w1(w1[e], H, I) for e in range(E)]
    w2_packed = [pack_w2(w2[e], H, I) for e in range(E)]

    out = np.zeros_like(x, dtype=np.float32)
    for r in range(rounds):
        in_maps = []
        for e in range(E):
            ix = chunked[e][r]
            xq_fn = quantize_ref(x[ix], float(w1_iscale[e]))
            s1 = 4.0 * float(w1_iscale[e]) * float(w1_wscale[e])
            c2 = s1 / (2.0 * float(w2_iscale[e]))
            sc = np.empty((128, 2), dtype=np.float32)
            sc[:, 0] = s1
            sc[:, 1] = c2
            in_maps.append({
                "xq": pack_xq(xq_fn, CP, colmap, H),
                "w1p": w1_packed[e],
                "w2p": w2_packed[e],
                "sc": sc,
            })

        kwargs = {"trace": True} if _maybe_enable_trace() else {}
        res = run_bass_kernel_spmd(nc, in_maps, core_ids=list(range(E)),
                                   **kwargs)
        LAST_RESULT = res

        for e in range(E):
            ix = chunked[e][r]
            yTe = res.results[e]["yT"]                   # [H, CP] f32 (y~^T)
            ye = np.ascontiguousarray(yTe[:, colmap[:len(ix)]].T)  # [cnt, H]
            s2 = 4.0 * float(w2_iscale[e]) * float(w2_wscale[e])
            out[ix] += (gate[ix, e] * s2)[:, None] * ye
    return out.astype(x.dtype)


# revision 7
# speedup vs baseline: 1.1901x; 1.1901x over previous
"""FP8 MoE (top-2, 8 experts) Trainium2 kernel.

Strategy (expert-parallel over 8 NeuronCores):
  - Host: compute per-token per-expert gate = sum(routing_weights where
    selected_experts == e).  Tokens with gate == 0 contribute exactly 0 to the
    reference output, so each expert only processes its selected tokens
    (~T*K/E each instead of T).
  - Host: quantize activations x -> fp8 e4m3fn exactly as the reference does,
    then map the e4m3fn grid onto Trainium's IEEE e4m3 grid (max 240 vs 448)
    by halving (exact: exponent decrement).  Weights (already e4m3fn values
    stored as f32) are halved the same way.  The compensating 4x lands in the
    dequant scales.
  - Device (per core = per expert): h~ = xq_t @ w1_t^T via DoubleRow fp8
    matmuls (features on PSUM partitions, tokens on the moving free dim);
    aq = fp8(clip(silu(s1*g~) * c2*u~, +-224)); y~ = aq_t @ w2_t^T with
    DoubleRow.  DR adds ~1e-4 noise; total rel_fro ~3e-3, well inside the
    2e-2 gate.
  - Tokens live in <=504-wide column slots at 512-aligned offsets: 512-wide
    matmuls (moving free dim 2*512 = fp8 max, full PSUM bank) measure ~8ns/MM
    slower than narrower ones, and 512-aligned slot offsets keep the fp8
    DoubleRow pair-stride 16B-aligned without rounding the token capacity up.
  - DMA: xq rides the Scalar (Activation) HWDGE queue in parallel with the
    weight stream on the Sync queue; the per-partition-tiny sc transfer goes
    last so it cannot clog the prologue.
  - Host: out[tok] += gate * s2 * y~  (s2 = 4*iscale2*wscale2), experts in
    ascending order like the reference loop.
"""

import os

import numpy as np
import ml_dtypes

import concourse.mybir as mybir
from concourse import bacc
from concourse.tile import TileContext
from concourse.bass_utils import run_bass_kernel_spmd

FP8_MAX = 448.0
E4 = mybir.dt.float8e4
F32 = mybir.dt.float32
E4NP = ml_dtypes.float8_e4m3        # TRN fp8 (IEEE, max 240)
FNNP = ml_dtypes.float8_e4m3fn      # OCP fp8 (max 448) — the reference format

# Problem sizes (hardcoded; harness contract).
T, H, I, E, TOPK = 4096, 2048, 4096, 8, 2

SLOT_W = 504        # max tokens per column slot (see module docstring)
SLOT_STRIDE = 512   # slot column offsets are 512-aligned

# Module global holding the most recent BassKernelResults (for test harness).
LAST_RESULT = None

# Compiled-kernel cache keyed by the slot layout so repeated kernel() calls
# with the same routing shape skip the neuronxcc compile.
_NC_CACHE = {}


# --------------------------------------------------------------------------
# Device kernel builder (shape-generic so it can be smoke-tested small).
# --------------------------------------------------------------------------

def build_nc(CP, slots, h=H, i_dim=I, n_cores=8, mm1_dr=True, mm2_dr=True):
    """Two fp8 GEMMs + SiLU-gate epilogue for one expert.

    Tokens occupy column slots [off, off+sz) of a CP-wide layout.

    Inputs (per core):
      xq  [KT1, 128, 2*CP] fp8e4 : xq^T tiled (k-chunk, pair, token-col)
      w1p [M1, 128, KT1*256] fp8e4 : w1^T tiled per output-feature block
      w2p [M2, 128, KT2*256] fp8e4
      sc  [128, 2] f32 : col0 = s1 = 4*is1*ws1, col1 = c2 = s1/(2*is2)
    Output:
      yT  [h, CP] f32 : y~^T (caller applies s2 = 4*is2*ws2 and the gate)
    """
    assert h % 256 == 0 and i_dim % 256 == 0
    assert CP % 16 == 0
    nslices = [slice(off, off + sz) for off, sz in slots]
    for off, sz in slots:
        assert off % 16 == 0 and 0 < sz <= 512 and off + sz <= CP
    KT1 = h // 256          # mm1 k-chunks (256 deep each w/ DoubleRow)
    KT2 = i_dim // 256      # mm2 k-chunks
    MG = i_dim // 128       # gate-half feature tiles (m and m+MG pair up)
    M1 = 2 * MG             # w1 output tiles
    M2 = h // 128           # w2 output tiles
    DR = mybir.MatmulPerfMode.DoubleRow
    mult = mybir.AluOpType.mult

    nc = bacc.Bacc("TRN2", target_bir_lowering=False, debug=False,
                   num_devices=n_cores)
    xq = nc.dram_tensor("xq", [KT1, 128, 2 * CP], E4, kind="ExternalInput")
    w1p = nc.dram_tensor("w1p", [M1, 128, KT1 * 256], E4, kind="ExternalInput")
    w2p = nc.dram_tensor("w2p", [M2, 128, KT2 * 256], E4, kind="ExternalInput")
    sc = nc.dram_tensor("sc", [128, 2], F32, kind="ExternalInput")
    yT = nc.dram_tensor("yT", [h, CP], F32, kind="ExternalOutput")

    with TileContext(nc) as tc:
        with (
            tc.tile_pool(name="cpool", bufs=1) as cpool,
            tc.tile_pool(name="xqpool", bufs=1) as xqpool,
            tc.tile_pool(name="aqpool", bufs=1) as aqpool,
            tc.tile_pool(name="w1pool", bufs=6) as w1pool,
            tc.tile_pool(name="w2pool", bufs=4) as w2pool,
            tc.tile_pool(name="eppool", bufs=3) as eppool,
            tc.tile_pool(name="ypool", bufs=4) as ypool,
            tc.tile_pool(name="psA", bufs=2, space="PSUM") as psA,
            tc.tile_pool(name="psB", bufs=2, space="PSUM") as psB,
            tc.tile_pool(name="psY", bufs=3, space="PSUM") as psY,
        ):
            # PE warmup: dependency-free matmuls on a memset tile start the
            # HAM clock ramp while the first input DMAs are in flight.  Kept
            # short: the first real matmul can start as soon as xq chunk 0
            # lands (~8 us), and in-order PE would stall behind extra warmup.
            wrm = cpool.tile([128, 512], E4, name="wrm")
            nc.vector.memset(wrm, 0)
            pwrm = psY.tile([128, 512], F32, name="pwrm", bufs=1, tag="pwrm")
            for _ in range(3):
                nc.tensor.matmul(pwrm, wrm[:, 0:128], wrm, start=True,
                                 stop=True)

            def load_w1(j):
                wt = w1pool.tile([128, KT1 * 256], E4, name=f"w1t{j}",
                                 tag="w1")
                nc.sync.dma_start(out=wt, in_=w1p.ap()[j])
                return wt

            # Prologue schedule: the first matmul group needs w1 tiles
            # (j=0, MG) plus ALL xq k-chunks.  Split those across the two
            # HWDGE queues so they stream in parallel: Sync carries the two
            # w1 tiles + the tail xq chunks, Scalar carries the head xq
            # chunks.  The sc transfer (128 rows x 8 B = tiny packets) goes
            # last on Scalar where it cannot delay anything.
            w1_first = (load_w1(0), load_w1(MG))
            xq_ts = []
            n_scalar_xq = (5 * KT1) // 8
            for k in range(KT1):
                xqk = xqpool.tile([128, 2, CP], E4, name=f"xqk{k}",
                                  tag=f"xqk{k}")
                eng = nc.scalar if k < n_scalar_xq else nc.sync
                eng.dma_start(out=xqk, in_=xq.ap()[k].rearrange(
                    "p (i n) -> p i n", i=2))
                xq_ts.append(xqk)
            sc_t = cpool.tile([128, 2], F32, name="sc_t")
            nc.scalar.dma_start(out=sc_t, in_=sc.ap())
            s1_ap = sc_t[:, 0:1]
            c2_ap = sc_t[:, 1:2]
            aq_t = aqpool.tile([128, 2 * KT2, CP], E4, name="aq_t")

            # ---- mm1 + gated epilogue: aq^T[i_dim, CP] in fp8 ----
            def mm1_epilogue(jg, nsl, pg, pu):
                nt_sz = nsl.stop - nsl.start
                tg = eppool.tile([128, nt_sz], F32, name="tg", tag="tg")
                nc.scalar.activation(tg, pg,
                                     mybir.ActivationFunctionType.Silu,
                                     scale=s1_ap)
                v = eppool.tile([128, nt_sz], F32, name="v", tag="v")
                nc.vector.scalar_tensor_tensor(v, pu, c2_ap, tg,
                                               op0=mult, op1=mult)
                nc.vector.tensor_scalar(
                    aq_t[:, jg, nsl], v, 224.0, -224.0,
                    op0=mybir.AluOpType.min, op1=mybir.AluOpType.max)

            # jg = 0 runs k-major: each xq chunk feeds 4 matmuls (2 halves x
            # 2 slots, interleaved PSUM accumulation groups) the moment it
            # lands, so the PE tracks the xq DMA stream instead of idling
            # until the whole 2 MB transfer completes.
            if mm1_dr is True and len(nslices) <= 2:
                wg, wu = w1_first
                pgs = [psA.tile([128, s.stop - s.start], F32, name="pg",
                                tag="pg") for s in nslices]
                pus = [psB.tile([128, s.stop - s.start], F32, name="pu",
                                tag="pu") for s in nslices]
                for k in range(KT1):
                    for wtile, ptiles in ((wg, pgs), (wu, pus)):
                        lh = wtile[:, k * 256:(k + 1) * 256].rearrange(
                            "p (i m) -> p i m", i=2)
                        for nsl, ptile in zip(nslices, ptiles):
                            nc.tensor.matmul(
                                ptile, lh, xq_ts[k][:, :, nsl],
                                start=(k == 0), stop=(k == KT1 - 1),
                                perf_mode=DR)
                for nsl, pg, pu in zip(nslices, pgs, pus):
                    mm1_epilogue(0, nsl, pg, pu)
                jg_start = 1
            else:
                jg_start = 0

            for jg in range(jg_start, MG):
                wg, wu = (w1_first if jg == 0
                          else (load_w1(jg), load_w1(jg + MG)))
                for nsl in nslices:
                    nt_sz = nsl.stop - nsl.start
                    pg = psA.tile([128, nt_sz], F32, name="pg", tag="pg")
                    pu = psB.tile([128, nt_sz], F32, name="pu", tag="pu")
                    for half, (wtile, ptile) in enumerate(((wg, pg),
                                                          (wu, pu))):
                        use_dr = (mm1_dr is True
                                  or (mm1_dr == "g" and half == 0)
                                  or (mm1_dr == "u" and half == 1))
                        if use_dr:
                            for k in range(KT1):
                                lh = wtile[:, k * 256:(k + 1) * 256].rearrange(
                                    "p (i m) -> p i m", i=2)
                                rx = xq_ts[k][:, :, nsl]
                                nc.tensor.matmul(
                                    ptile, lh, rx, start=(k == 0),
                                    stop=(k == KT1 - 1), perf_mode=DR)
                        else:
                            for c in range(2 * KT1):
                                lh = wtile[:, c * 128:(c + 1) * 128]
                                rx = xq_ts[c // 2][:, c % 2, nsl]
                                nc.tensor.matmul(
                                    ptile, lh, rx, start=(c == 0),
                                    stop=(c == 2 * KT1 - 1))
                    mm1_epilogue(jg, nsl, pg, pu)

            # ---- mm2: y~^T[h, CP] ----
            for m in range(M2):
                w2t = w2pool.tile([128, KT2 * 256], E4, name="w2t", tag="w2")
                nc.sync.dma_start(out=w2t, in_=w2p.ap()[m])
                for nsl in nslices:
                    nt_sz = nsl.stop - nsl.start
                    py = psY.tile([128, nt_sz], F32, name="py", tag="py")
                    if mm2_dr:
                        for k in range(KT2):
                            lw = w2t[:, k * 256:(k + 1) * 256].rearrange(
                                "p (i m) -> p i m", i=2)
                            ra = aq_t[:, 2 * k:2 * k + 2, nsl]
                            nc.tensor.matmul(py, lw, ra, start=(k == 0),
                                             stop=(k == KT2 - 1), perf_mode=DR)
                    else:
                        for c in range(2 * KT2):
                            lw = w2t[:, c * 128:(c + 1) * 128]
                            ra = aq_t[:, c, nsl]
                            nc.tensor.matmul(py, lw, ra, start=(c == 0),
                                             stop=(c == 2 * KT2 - 1))
                    yt = ypool.tile([128, nt_sz], F32, name="yt", tag="yt")
                    if m == M2 - 1 and nsl is nslices[-1]:
                        # tail: quarter the last copy+store across both HWDGE
                        # queues so the final DMA chases a quarter-size copy.
                        qw = (nt_sz // 4 + 7) // 8 * 8
                        cuts = [0, qw, 2 * qw, 3 * qw, nt_sz]
                        for qi in range(4):
                            hsl = slice(cuts[qi], cuts[qi + 1])
                            nc.vector.tensor_copy(out=yt[:, hsl],
                                                  in_=py[:, hsl])
                            qeng = nc.scalar if qi % 2 == 0 else nc.sync
                            qeng.dma_start(
                                out=yT.ap()[m * 128:(m + 1) * 128,
                                            nsl.start + hsl.start:
                                            nsl.start + hsl.stop],
                                in_=yt[:, hsl])
                    else:
                        nc.vector.tensor_copy(out=yt, in_=py)
                        nc.sync.dma_start(
                            out=yT.ap()[m * 128:(m + 1) * 128, nsl], in_=yt)
    nc.compile()
    return nc


# --------------------------------------------------------------------------
# Host-side packing
# --------------------------------------------------------------------------

def _halve_to_trn(q_fn_f32):
    """e4m3fn values (held in f32) -> TRN e4m3 at half scale (exact)."""
    return (q_fn_f32.astype(np.float32) * 0.5).astype(E4NP)


def pack_w1(w1_e, h, i_dim):
    """w1_e [2I, H] f32 (e4m3fn values) -> [M1, 128, KT1*256] TRN fp8."""
    M1, KT1 = (2 * i_dim) // 128, h // 256
    q = _halve_to_trn(w1_e)
    t = q.reshape(M1, 128, KT1, 2, 128)            # [m, mm, k, i, p]
    t = np.ascontiguousarray(t.transpose(0, 4, 2, 3, 1))  # [m, p, k, i, mm]
    return t.reshape(M1, 128, KT1 * 256)


def pack_w2(w2_e, h, i_dim):
    """w2_e [H, I] f32 (e4m3fn values) -> [M2, 128, KT2*256] TRN fp8."""
    M2, KT2 = h // 128, i_dim // 256
    q = _halve_to_trn(w2_e)
    t = q.reshape(M2, 128, KT2, 2, 128)
    t = np.ascontiguousarray(t.transpose(0, 4, 2, 3, 1))
    return t.reshape(M2, 128, KT2 * 256)


def quantize_ref(xg, iscale):
    """Exactly the reference's _to_fp8(x/iscale), values in f32."""
    q = np.clip(xg.astype(np.float32) / iscale, -FP8_MAX, FP8_MAX)
    return q.astype(FNNP).astype(np.float32)


def pack_xq(xq_fn_f32, CP, colmap, h):
    """Quantized tokens [cnt, H] (e4m3fn values) -> [KT1, 128, 2*CP]."""
    KT1 = h // 256
    cnt = xq_fn_f32.shape[0]
    zq = np.zeros((CP, h), dtype=E4NP)
    zq[colmap[:cnt]] = _halve_to_trn(xq_fn_f32)
    xqT = np.ascontiguousarray(zq.T)               # [h, CP]
    t = xqT.reshape(KT1, 2, 128, CP)               # [k, i, p, n]
    t = np.ascontiguousarray(t.transpose(0, 2, 1, 3))  # [k, p, i, n]
    return t.reshape(KT1, 128, 2 * CP)


def choose_capacity(max_cnt):
    """Slot layout for max_cnt tokens: (C, CP, slots, colmap).

    slots are (offset, width) with width <= SLOT_W and 512-aligned offsets;
    colmap[i] is the column index of the i-th packed token.
    """
    C = max(max_cnt, 8)
    nslot = -(-C // SLOT_W)
    slots = []
    left = C
    for i in range(nslot):
        w = min(SLOT_W, left)
        slots.append((SLOT_STRIDE * i, w))
        left -= w
    CP = -(-(slots[-1][0] + slots[-1][1]) // 16) * 16
    colmap = np.concatenate([np.arange(off, off + w) for off, w in slots])
    return C, CP, slots, colmap


def _maybe_enable_trace():
    """NTFF tracing (MOE_TRACE=1): install the antenv.axon_hooks shim this
    image lacks so run_bass_kernel_spmd(trace=True) works under axon."""
    if not os.environ.get("MOE_TRACE"):
        return False
    try:
        import antenv.axon_hooks  # noqa: F401
    except ImportError:
        import sys
        import types
        mod = types.ModuleType("antenv.axon_hooks")
        mod._hook = None
        mod.set_axon_ntff_profile_hook = lambda h: setattr(mod, "_hook", h)
        mod.get_axon_ntff_profile_hook = lambda: mod._hook
        sys.modules["antenv.axon_hooks"] = mod
        try:
            from trn_agent_boot.trn_boot import _ntff_profile_via_ctypes
            mod._hook = _ntff_profile_via_ctypes("/opt/axon/libaxon_pjrt.so")
        except Exception:
            return False
    return True


# --------------------------------------------------------------------------
# Entry point
# --------------------------------------------------------------------------

def kernel(x, selected_experts, routing_weights, w1, w2,
           w1_iscale, w2_iscale, w1_wscale, w2_wscale):
    global LAST_RESULT
    x = np.asarray(x)
    sel = np.asarray(selected_experts)
    rw = np.asarray(routing_weights).astype(np.float32)
    w1 = np.asarray(w1)
    w2 = np.asarray(w2)
    w1_iscale = np.asarray(w1_iscale, dtype=np.float32)
    w2_iscale = np.asarray(w2_iscale, dtype=np.float32)
    w1_wscale = np.asarray(w1_wscale, dtype=np.float32)
    w2_wscale = np.asarray(w2_wscale, dtype=np.float32)

    t_dim = x.shape[0]
    # gate[t, e] = sum_k rw[t, k] * (sel[t, k] == e)
    gate = np.zeros((t_dim, E), dtype=np.float32)
    rows = np.arange(t_dim)
    for kk in range(sel.shape[1]):
        np.add.at(gate, (rows, sel[:, kk]), rw[:, kk])

    idxs = [np.flatnonzero(gate[:, e] != 0.0) for e in range(E)]
    counts = [len(ix) for ix in idxs]
    # SBUF caps the per-round token capacity; very skewed routing falls back
    # to multiple rounds over token chunks (key-0 inputs need one round).
    MAX_C = 2016
    rounds = max(1, -(-max(counts) // MAX_C))
    chunked = [np.array_split(ix, rounds) for ix in idxs]
    C, CP, slots, colmap = choose_capacity(
        max(len(ch) for chs in chunked for ch in chs))

    mm1_dr = {"1": True, "g": "g", "u": "u", "0": False}.get(
        os.environ.get("MOE_MM1_DR", ""), True)
    key = (CP, tuple(slots), mm1_dr)
    nc = _NC_CACHE.get(key)
    if nc is None:
        nc = _NC_CACHE[key] = build_nc(CP, slots, mm1_dr=mm1_dr)
    w1_packed = [pack_w1(w1[e], H, I) for e in range(E)]
    w2_packed = [pack_w2(w2[e], H, I) for e in range(E)]

    out = np.zeros_like(x, dtype=np.float32)
    for r in range(rounds):
        in_maps = []
        for e in range(E):
            ix = chunked[e][r]
            xq_fn = quantize_ref(x[ix], float(w1_iscale[e]))
            s1 = 4.0 * float(w1_iscale[e]) * float(w1_wscale[e])
            c2 = s1 / (2.0 * float(w2_iscale[e]))
            sc = np.empty((128, 2), dtype=np.float32)
            sc[:, 0] = s1
            sc[:, 1] = c2
            in_maps.append({
                "xq": pack_xq(xq_fn, CP, colmap, H),
                "w1p": w1_packed[e],
                "w2p": w2_packed[e],
                "sc": sc,
            })

        kwargs = {"trace": True} if _maybe_enable_trace() else {}
        res = run_bass_kernel_spmd(nc, in_maps, core_ids=list(range(E)),
                                   **kwargs)
        LAST_RESULT = res

        for e in range(E):
            ix = chunked[e][r]
            yTe = res.results[e]["yT"]                   # [H, CP] f32 (y~^T)
            ye = np.ascontiguousarray(yTe[:, colmap[:len(ix)]].T)  # [cnt, H]
            s2 = 4.0 * float(w2_iscale[e]) * float(w2_wscale[e])
            out[ix] += (gate[ix, e] * s2)[:, None] * ye
    return out.astype(x.dtype)


# revision 10
# speedup vs baseline: 1.1914x; 1.0011x over previous
"""FP8 MoE (top-2, 8 experts) Trainium2 kernel.

Strategy (expert-parallel over 8 NeuronCores):
  - Host: compute per-token per-expert gate = sum(routing_weights where
    selected_experts == e).  Tokens with gate == 0 contribute exactly 0 to the
    reference output, so each expert only processes its selected tokens
    (~T*K/E each instead of T).
  - Host: quantize activations x -> fp8 e4m3fn exactly as the reference does,
    then map the e4m3fn grid onto Trainium's IEEE e4m3 grid (max 240 vs 448)
    by halving (exact: exponent decrement).  Weights (already e4m3fn values
    stored as f32) are halved the same way.  The compensating 4x lands in the
    dequant scales.
  - Device (per core = per expert): h~ = xq_t @ w1_t^T via DoubleRow fp8
    matmuls (features on PSUM partitions, tokens on the moving free dim);
    aq = fp8(clip(silu(s1*g~) * c2*u~, +-224)); y~ = aq_t @ w2_t^T with
    DoubleRow.  DR adds ~1e-4 noise; total rel_fro ~3e-3, well inside the
    2e-2 gate.
  - Tokens live in <=504-wide column slots at 512-aligned offsets: 512-wide
    matmuls (moving free dim 2*512 = fp8 max, full PSUM bank) measure ~8ns/MM
    slower than narrower ones, and 512-aligned slot offsets keep the fp8
    DoubleRow pair-stride 16B-aligned without rounding the token capacity up.
  - DMA: xq rides the Scalar (Activation) HWDGE queue in parallel with the
    weight stream on the Sync queue; the per-partition-tiny sc transfer goes
    last so it cannot clog the prologue.
  - Host: out[tok] += gate * s2 * y~  (s2 = 4*iscale2*wscale2), experts in
    ascending order like the reference loop.
"""

import os

import numpy as np
import ml_dtypes

import concourse.mybir as mybir
from concourse import bacc
from concourse.tile import TileContext
from concourse.bass_utils import run_bass_kernel_spmd

FP8_MAX = 448.0
E4 = mybir.dt.float8e4
F32 = mybir.dt.float32
E4NP = ml_dtypes.float8_e4m3        # TRN fp8 (IEEE, max 240)
FNNP = ml_dtypes.float8_e4m3fn      # OCP fp8 (max 448) — the reference format

# Problem sizes (hardcoded; harness contract).
T, H, I, E, TOPK = 4096, 2048, 4096, 8, 2

SLOT_W = 504        # max tokens per column slot (see module docstring)
SLOT_STRIDE = 512   # slot column offsets are 512-aligned

# Module global holding the most recent BassKernelResults (for test harness).
LAST_RESULT = None

# Compiled-kernel cache keyed by the slot layout so repeated kernel() calls
# with the same routing shape skip the neuronxcc compile.
_NC_CACHE = {}


# --------------------------------------------------------------------------
# Device kernel builder (shape-generic so it can be smoke-tested small).
# --------------------------------------------------------------------------

def build_nc(CP, slots, h=H, i_dim=I, n_cores=8, mm1_dr=True, mm2_dr=True):
    """Two fp8 GEMMs + SiLU-gate epilogue for one expert.

    Tokens occupy column slots [off, off+sz) of a CP-wide layout.

    Inputs (per core):
      xq  [KT1, 128, 2*CP] fp8e4 : xq^T tiled (k-chunk, pair, token-col)
      w1p [M1, 128, KT1*256] fp8e4 : w1^T tiled per output-feature block
      w2p [M2, 128, KT2*256] fp8e4
      sc  [128, 2] f32 : col0 = s1 = 4*is1*ws1, col1 = c2 = s1/(2*is2)
    Output:
      yT  [h, CP] f32 : y~^T (caller applies s2 = 4*is2*ws2 and the gate)
    """
    assert h % 256 == 0 and i_dim % 256 == 0
    assert CP % 16 == 0
    nslices = [slice(off, off + sz) for off, sz in slots]
    for off, sz in slots:
        assert off % 16 == 0 and 0 < sz <= 512 and off + sz <= CP
    KT1 = h // 256          # mm1 k-chunks (256 deep each w/ DoubleRow)
    KT2 = i_dim // 256      # mm2 k-chunks
    MG = i_dim // 128       # gate-half feature tiles (m and m+MG pair up)
    M1 = 2 * MG             # w1 output tiles
    M2 = h // 128           # w2 output tiles
    DR = mybir.MatmulPerfMode.DoubleRow
    mult = mybir.AluOpType.mult

    nc = bacc.Bacc("TRN2", target_bir_lowering=False, debug=False,
                   num_devices=n_cores)
    xq = nc.dram_tensor("xq", [KT1, 128, 2 * CP], E4, kind="ExternalInput")
    w1p = nc.dram_tensor("w1p", [M1, 128, KT1 * 256], E4, kind="ExternalInput")
    w2p = nc.dram_tensor("w2p", [M2, 128, KT2 * 256], E4, kind="ExternalInput")
    sc = nc.dram_tensor("sc", [128, 2], F32, kind="ExternalInput")
    yT = nc.dram_tensor("yT", [h, CP], F32, kind="ExternalOutput")

    with TileContext(nc) as tc:
        with (
            tc.tile_pool(name="cpool", bufs=1) as cpool,
            tc.tile_pool(name="xqpool", bufs=1) as xqpool,
            tc.tile_pool(name="aqpool", bufs=1) as aqpool,
            tc.tile_pool(name="w1pool", bufs=6) as w1pool,
            tc.tile_pool(name="w2pool", bufs=4) as w2pool,
            tc.tile_pool(name="eppool", bufs=3) as eppool,
            tc.tile_pool(name="ypool", bufs=4) as ypool,
            tc.tile_pool(name="psA", bufs=2, space="PSUM") as psA,
            tc.tile_pool(name="psB", bufs=2, space="PSUM") as psB,
            tc.tile_pool(name="psY", bufs=3, space="PSUM") as psY,
        ):
            # PE warmup: one dependency-free matmul on a memset tile starts
            # the HAM clock ramp while the first input DMAs are in flight.
            # The ramp is a free-running ~3.4us wall-time window, so extra
            # warmup matmuls only delay the first real matmul (in-order PE)
            # once xq chunk 0 has landed (~7 us).
            wrm = cpool.tile([128, 512], E4, name="wrm")
            nc.vector.memset(wrm, 0)
            pwrm = psY.tile([128, 512], F32, name="pwrm", bufs=1, tag="pwrm")
            nc.tensor.matmul(pwrm, wrm[:, 0:128], wrm, start=True, stop=True)

            def load_w1(j):
                wt = w1pool.tile([128, KT1 * 256], E4, name=f"w1t{j}",
                                 tag="w1")
                nc.sync.dma_start(out=wt, in_=w1p.ap()[j])
                return wt

            # Prologue schedule: the first matmul group needs w1 tiles
            # (j=0, MG) plus ALL xq k-chunks.  Split those across the two
            # HWDGE queues so they stream in parallel: Sync carries the two
            # w1 tiles + the tail xq chunks, Scalar carries the head xq
            # chunks.  The sc transfer (128 rows x 8 B = tiny packets) goes
            # last on Scalar where it cannot delay anything.
            w1_first = (load_w1(0), load_w1(MG))
            xq_ts = []
            for k in range(KT1):
                xqk = xqpool.tile([128, 2, CP], E4, name=f"xqk{k}",
                                  tag=f"xqk{k}")
                # parity-interleave across the two queues: Scalar starts
                # clean (k0 lands first) while Sync leads with the two w1
                # tiles, so chunks arrive roughly in consumption order.
                eng = nc.scalar if k % 2 == 0 else nc.sync
                eng.dma_start(out=xqk, in_=xq.ap()[k].rearrange(
                    "p (i n) -> p i n", i=2))
                xq_ts.append(xqk)
            sc_t = cpool.tile([128, 2], F32, name="sc_t")
            nc.scalar.dma_start(out=sc_t, in_=sc.ap())
            s1_ap = sc_t[:, 0:1]
            c2_ap = sc_t[:, 1:2]
            aq_t = aqpool.tile([128, 2 * KT2, CP], E4, name="aq_t")

            # ---- mm1 + gated epilogue: aq^T[i_dim, CP] in fp8 ----
            def mm1_epilogue(jg, nsl, pg, pu):
                nt_sz = nsl.stop - nsl.start
                tg = eppool.tile([128, nt_sz], F32, name="tg", tag="tg")
                nc.scalar.activation(tg, pg,
                                     mybir.ActivationFunctionType.Silu,
                                     scale=s1_ap)
                v = eppool.tile([128, nt_sz], F32, name="v", tag="v")
                nc.vector.scalar_tensor_tensor(v, pu, c2_ap, tg,
                                               op0=mult, op1=mult)
                nc.vector.tensor_scalar(
                    aq_t[:, jg, nsl], v, 224.0, -224.0,
                    op0=mybir.AluOpType.min, op1=mybir.AluOpType.max)

            # jg = 0 runs k-major: each xq chunk feeds 4 matmuls (2 halves x
            # 2 slots, interleaved PSUM accumulation groups) the moment it
            # lands, so the PE tracks the xq DMA stream instead of idling
            # until the whole 2 MB transfer completes.
            if mm1_dr is True and len(nslices) <= 2:
                wg, wu = w1_first
                pgs = [psA.tile([128, s.stop - s.start], F32, name="pg",
                                tag="pg") for s in nslices]
                pus = [psB.tile([128, s.stop - s.start], F32, name="pu",
                                tag="pu") for s in nslices]
                for k in range(KT1):
                    for wtile, ptiles in ((wg, pgs), (wu, pus)):
                        lh = wtile[:, k * 256:(k + 1) * 256].rearrange(
                            "p (i m) -> p i m", i=2)
                        for nsl, ptile in zip(nslices, ptiles):
                            nc.tensor.matmul(
                                ptile, lh, xq_ts[k][:, :, nsl],
                                start=(k == 0), stop=(k == KT1 - 1),
                                perf_mode=DR)
                for nsl, pg, pu in zip(nslices, pgs, pus):
                    mm1_epilogue(0, nsl, pg, pu)
                jg_start = 1
            else:
                jg_start = 0

            for jg in range(jg_start, MG):
                wg, wu = (w1_first if jg == 0
                          else (load_w1(jg), load_w1(jg + MG)))
                for nsl in nslices:
                    nt_sz = nsl.stop - nsl.start
                    pg = psA.tile([128, nt_sz], F32, name="pg", tag="pg")
                    pu = psB.tile([128, nt_sz], F32, name="pu", tag="pu")
                    for half, (wtile, ptile) in enumerate(((wg, pg),
                                                          (wu, pu))):
                        use_dr = (mm1_dr is True
                                  or (mm1_dr == "g" and half == 0)
                                  or (mm1_dr == "u" and half == 1))
                        if use_dr:
                            for k in range(KT1):
                                lh = wtile[:, k * 256:(k + 1) * 256].rearrange(
                                    "p (i m) -> p i m", i=2)
                                rx = xq_ts[k][:, :, nsl]
                                nc.tensor.matmul(
                                    ptile, lh, rx, start=(k == 0),
                                    stop=(k == KT1 - 1), perf_mode=DR)
                        else:
                            for c in range(2 * KT1):
                                lh = wtile[:, c * 128:(c + 1) * 128]
                                rx = xq_ts[c // 2][:, c % 2, nsl]
                                nc.tensor.matmul(
                                    ptile, lh, rx, start=(c == 0),
                                    stop=(c == 2 * KT1 - 1))
                    mm1_epilogue(jg, nsl, pg, pu)

            # ---- mm2: y~^T[h, CP] ----
            for m in range(M2):
                w2t = w2pool.tile([128, KT2 * 256], E4, name="w2t", tag="w2")
                nc.sync.dma_start(out=w2t, in_=w2p.ap()[m])
                for nsl in nslices:
                    nt_sz = nsl.stop - nsl.start
                    py = psY.tile([128, nt_sz], F32, name="py", tag="py")
                    if mm2_dr:
                        for k in range(KT2):
                            lw = w2t[:, k * 256:(k + 1) * 256].rearrange(
                                "p (i m) -> p i m", i=2)
                            ra = aq_t[:, 2 * k:2 * k + 2, nsl]
                            nc.tensor.matmul(py, lw, ra, start=(k == 0),
                                             stop=(k == KT2 - 1), perf_mode=DR)
                    else:
                        for c in range(2 * KT2):
                            lw = w2t[:, c * 128:(c + 1) * 128]
                            ra = aq_t[:, c, nsl]
                            nc.tensor.matmul(py, lw, ra, start=(c == 0),
                                             stop=(c == 2 * KT2 - 1))
                    yt = ypool.tile([128, nt_sz], F32, name="yt", tag="yt")
                    if m == M2 - 1 and nsl is nslices[-1]:
                        # tail: halve the last copy+store across both HWDGE
                        # queues (one 624ns trigger each) so the final DMA
                        # chases a half-size copy.
                        hw_ = (nt_sz // 2 + 7) // 8 * 8
                        for qeng, hsl in ((nc.scalar, slice(0, hw_)),
                                          (nc.sync, slice(hw_, nt_sz))):
                            nc.vector.tensor_copy(out=yt[:, hsl],
                                                  in_=py[:, hsl])
                            qeng.dma_start(
                                out=yT.ap()[m * 128:(m + 1) * 128,
                                            nsl.start + hsl.start:
                                            nsl.start + hsl.stop],
                                in_=yt[:, hsl])
                    else:
                        nc.vector.tensor_copy(out=yt, in_=py)
                        nc.sync.dma_start(
                            out=yT.ap()[m * 128:(m + 1) * 128, nsl], in_=yt)
    nc.compile()
    return nc


# --------------------------------------------------------------------------
# Host-side packing
# --------------------------------------------------------------------------

def _halve_to_trn(q_fn_f32):
    """e4m3fn values (held in f32) -> TRN e4m3 at half scale (exact)."""
    return (q_fn_f32.astype(np.float32) * 0.5).astype(E4NP)


def pack_w1(w1_e, h, i_dim):
    """w1_e [2I, H] f32 (e4m3fn values) -> [M1, 128, KT1*256] TRN fp8."""
    M1, KT1 = (2 * i_dim) // 128, h // 256
    q = _halve_to_trn(w1_e)
    t = q.reshape(M1, 128, KT1, 2, 128)            # [m, mm, k, i, p]
    t = np.ascontiguousarray(t.transpose(0, 4, 2, 3, 1))  # [m, p, k, i, mm]
    return t.reshape(M1, 128, KT1 * 256)


def pack_w2(w2_e, h, i_dim):
    """w2_e [H, I] f32 (e4m3fn values) -> [M2, 128, KT2*256] TRN fp8."""
    M2, KT2 = h // 128, i_dim // 256
    q = _halve_to_trn(w2_e)
    t = q.reshape(M2, 128, KT2, 2, 128)
    t = np.ascontiguousarray(t.transpose(0, 4, 2, 3, 1))
    return t.reshape(M2, 128, KT2 * 256)


def quantize_ref(xg, iscale):
    """Exactly the reference's _to_fp8(x/iscale), values in f32."""
    q = np.clip(xg.astype(np.float32) / iscale, -FP8_MAX, FP8_MAX)
    return q.astype(FNNP).astype(np.float32)


def pack_xq(xq_fn_f32, CP, colmap, h):
    """Quantized tokens [cnt, H] (e4m3fn values) -> [KT1, 128, 2*CP]."""
    KT1 = h // 256
    cnt = xq_fn_f32.shape[0]
    zq = np.zeros((CP, h), dtype=E4NP)
    zq[colmap[:cnt]] = _halve_to_trn(xq_fn_f32)
    xqT = np.ascontiguousarray(zq.T)               # [h, CP]
    t = xqT.reshape(KT1, 2, 128, CP)               # [k, i, p, n]
    t = np.ascontiguousarray(t.transpose(0, 2, 1, 3))  # [k, p, i, n]
    return t.reshape(KT1, 128, 2 * CP)


def choose_capacity(max_cnt):
    """Slot layout for max_cnt tokens: (C, CP, slots, colmap).

    slots are (offset, width) with width <= SLOT_W and 512-aligned offsets;
    colmap[i] is the column index of the i-th packed token.
    """
    C = max(max_cnt, 8)
    nslot = -(-C // SLOT_W)
    slots = []
    left = C
    for i in range(nslot):
        w = min(SLOT_W, left)
        slots.append((SLOT_STRIDE * i, w))
        left -= w
    CP = -(-(slots[-1][0] + slots[-1][1]) // 16) * 16
    colmap = np.concatenate([np.arange(off, off + w) for off, w in slots])
    return C, CP, slots, colmap


def _maybe_enable_trace():
    """NTFF tracing (MOE_TRACE=1): install the antenv.axon_hooks shim this
    image lacks so run_bass_kernel_spmd(trace=True) works under axon."""
    if not os.environ.get("MOE_TRACE"):
        return False
    try:
        import antenv.axon_hooks  # noqa: F401
    except ImportError:
        import sys
        import types
        mod = types.ModuleType("antenv.axon_hooks")
        mod._hook = None
        mod.set_axon_ntff_profile_hook = lambda h: setattr(mod, "_hook", h)
        mod.get_axon_ntff_profile_hook = lambda: mod._hook
        sys.modules["antenv.axon_hooks"] = mod
        try:
            from trn_agent_boot.trn_boot import _ntff_profile_via_ctypes
            mod._hook = _ntff_profile_via_ctypes("/opt/axon/libaxon_pjrt.so")
        except Exception:
            return False
    return True


# --------------------------------------------------------------------------
# Entry point
# --------------------------------------------------------------------------

def kernel(x, selected_experts, routing_weights, w1, w2,
           w1_iscale, w2_iscale, w1_wscale, w2_wscale):
    global LAST_RESULT
    x = np.asarray(x)
    sel = np.asarray(selected_experts)
    rw = np.asarray(routing_weights).astype(np.float32)
    w1 = np.asarray(w1)
    w2 = np.asarray(w2)
    w1_iscale = np.asarray(w1_iscale, dtype=np.float32)
    w2_iscale = np.asarray(w2_iscale, dtype=np.float32)
    w1_wscale = np.asarray(w1_wscale, dtype=np.float32)
    w2_wscale = np.asarray(w2_wscale, dtype=np.float32)

    t_dim = x.shape[0]
    # gate[t, e] = sum_k rw[t, k] * (sel[t, k] == e)
    gate = np.zeros((t_dim, E), dtype=np.float32)
    rows = np.arange(t_dim)
    for kk in range(sel.shape[1]):
        np.add.at(gate, (rows, sel[:, kk]), rw[:, kk])

    idxs = [np.flatnonzero(gate[:, e] != 0.0) for e in range(E)]
    counts = [len(ix) for ix in idxs]
    # SBUF caps the per-round token capacity; very skewed routing falls back
    # to multiple rounds over token chunks (key-0 inputs need one round).
    MAX_C = 2016
    rounds = max(1, -(-max(counts) // MAX_C))
    chunked = [np.array_split(ix, rounds) for ix in idxs]
    C, CP, slots, colmap = choose_capacity(
        max(len(ch) for chs in chunked for ch in chs))

    mm1_dr = {"1": True, "g": "g", "u": "u", "0": False}.get(
        os.environ.get("MOE_MM1_DR", ""), True)
    key = (CP, tuple(slots), mm1_dr)
    nc = _NC_CACHE.get(key)
    if nc is None:
        nc = _NC_CACHE[key] = build_nc(CP, slots, mm1_dr=mm1_dr)
    w1_packed = [pack_w1(w1[e], H, I) for e in range(E)]
    w2_packed = [pack_w2(w2[e], H, I) for e in range(E)]

    out = np.zeros_like(x, dtype=np.float32)
    for r in range(rounds):
        in_maps = []
        for e in range(E):
            ix = chunked[e][r]
            xq_fn = quantize_ref(x[ix], float(w1_iscale[e]))
            s1 = 4.0 * float(w1_iscale[e]) * float(w1_wscale[e])
            c2 = s1 / (2.0 * float(w2_iscale[e]))
            sc = np.empty((128, 2), dtype=np.float32)
            sc[:, 0] = s1
            sc[:, 1] = c2
            in_maps.append({
                "xq": pack_xq(xq_fn, CP, colmap, H),
                "w1p": w1_packed[e],
                "w2p": w2_packed[e],
                "sc": sc,
            })

        kwargs = {"trace": True} if _maybe_enable_trace() else {}
        res = run_bass_kernel_spmd(nc, in_maps, core_ids=list(range(E)),
                                   **kwargs)
        LAST_RESULT = res

        for e in range(E):
            ix = chunked[e][r]
            yTe = res.results[e]["yT"]                   # [H, CP] f32 (y~^T)
            ye = np.ascontiguousarray(yTe[:, colmap[:len(ix)]].T)  # [cnt, H]
            s2 = 4.0 * float(w2_iscale[e]) * float(w2_wscale[e])
            out[ix] += (gate[ix, e] * s2)[:, None] * ye
    return out.astype(x.dtype)
